# revision 21
# baseline (speedup 1.0000x reference)
"""Trainium2 Bass kernel for nn_AttentionLayer (pre-conv + self-attention + final conv).

Sharding: 8 cores = 2 samples x 4 query-row chunks. Each core computes the
full pre-conv y for its sample (k/v need all N=9216 positions), attention for
its 26-row query window (24 own rows + 1 halo row each side for the final
3x3 conv), and the final conv for its 24 output rows.

Perf structure (v2): the kernel is a producer/consumer pipeline built around
the scalar-engine exp stream (the hard bottleneck: ~23M softmax elements at
1 elem/lane/cycle). The head streams input DMAs in 8-row chunks so the
pre-conv starts ~1us in, produces y/k/q/vt with PSUM at full width, and
splits PSUM evacuations between the scalar and vector engines. The attention
loop is software-pipelined one group ahead (emit order per group G:
PV(G-2), Energy(G), exp(G-1)) so the scalar engine never waits for the
tensor engine. Per-chunk epilogues (1/s broadcast, residual add, row pack)
run entirely on vector+DMA, and final-conv chunks are interleaved into the
stream as their input rows complete.
"""

import os
import hashlib
import shutil

import numpy as np
import ml_dtypes

BF16 = ml_dtypes.bfloat16
EPS = 1e-5

B, C, CQK, H, W = 2, 64, 16, 96, 96
N = H * W                       # 9216
QCH = 4                         # query chunks per sample
ROWS = H // QCH                 # 24 rows per core
LOCROWS = ROWS + 2              # 26 (with halo)
NLOC = LOCROWS * W              # 2496
HP, WP = H + 2, W + 2           # 98x98 padded frame
LOCP = LOCROWS + 2              # 28 padded local rows
NI_SIZES = [512, 512, 512, 512, 448]   # i-chunks over NLOC
JB = 128                        # j-block height
NJB = N // JB                   # 72
JG = 3                          # j-blocks per exp group (3-way tile_position)
NJG = NJB // JG                 # 24 groups per i-chunk
NG = NJG * len(NI_SIZES)        # 120 total groups
VB = C + 1                      # 65


# ---------------------------------------------------------------------------
# framework patches (self-contained)
# ---------------------------------------------------------------------------

def _apply_patches():
    import concourse.tile as tile
    import concourse.bass_utils as bu
    import concourse.bass2jax as b2j
    from concourse import mybir

    # 1) walrus in this env rejects >1-2 sync waits on the final Drain
    #    (CTRL_NO_STRUCT): split waits into single-wait nops.
    def _drain_and_barrier_split(self, tick_clock, wait_clock):
        nc = self.nc
        probe = nc.sync.nop()
        wait_clock.add_sem_waits(
            probe.ins, tile.ScopedClock({None: tick_clock.global_clock})
        )
        waits = list(probe.ins.sync_info.on_wait) if probe.ins.sync_info else []
        if probe.ins.sync_info is not None:
            probe.ins.sync_info.on_wait = []
        for w in waits[:-1]:
            nop = nc.sync.nop()
            if nop.ins.sync_info is None:
                nop.ins.sync_info = mybir.SyncInfo(on_wait=[w], on_update=[])
            else:
                nop.ins.sync_info.on_wait.append(w)
        drain_inst = nc.sync.drain()
        if waits:
            if drain_inst.ins.sync_info is None:
                drain_inst.ins.sync_info = mybir.SyncInfo(
                    on_wait=[waits[-1]], on_update=[]
                )
            else:
                drain_inst.ins.sync_info.on_wait.append(waits[-1])
        nc.all_engine_barrier()
        assert self.sems is not None
        popped = nc._tile_sem_poison_stack.pop()
        assert popped is self._sem_poison
        nc.clear_and_free_semaphores(list(self.sems.allocated().values()))
        nc.all_engine_barrier()

    tile.TileContext._drain_and_barrier = _drain_and_barrier_split

    # 2) NEFF disk cache keyed by BIR hash (compile is deterministic).
    cache_dir = os.path.join(os.path.dirname(os.path.abspath(__file__)),
                             ".neff_cache")
    try:
        os.makedirs(cache_dir, exist_ok=True)
    except OSError:
        cache_dir = None
    _orig_compile = bu.compile_bir_kernel

    def cached_compile(bir_json, tmpdir, neff_name="file.neff"):
        if cache_dir is None:
            return _orig_compile(bir_json, tmpdir, neff_name)
        h = hashlib.sha256(bir_json).hexdigest()[:24]
        cpath = os.path.join(cache_dir, f"{h}.neff")
        out = os.path.join(tmpdir, neff_name)
        if os.path.exists(cpath):
            shutil.copyfile(cpath, out)
            return out
        r = _orig_compile(bir_json, tmpdir, neff_name)
        try:
            shutil.copyfile(r, cpath)
        except OSError:
            pass
        return r

    bu.compile_bir_kernel = cached_compile
    b2j.compile_bir_kernel = cached_compile


def _split_excess_waits(nc, max_waits=1):
    """walrus in this env allows only a couple of sync-wait slots per
    instruction; move excess waits onto preceding same-engine NOPs."""
    from concourse import mybir
    idx = 0
    for f in nc.m.functions:
        for bb in f.blocks:
            new = []
            changed = False
            for inst in bb.instructions:
                si = inst.sync_info
                waits = list(si.on_wait) if si is not None and si.on_wait else []
                if len(waits) > max_waits:
                    changed = True
                    for w in waits[:-max_waits]:
                        idx += 1
                        nop = mybir.InstNoOp(name=f"wsplit_{idx}", ins=[], outs=[])
                        nop.engine = inst.engine
                        nop.sync_info = mybir.SyncInfo(on_wait=[w], on_update=[])
                        new.append(nop)
                    si.on_wait = waits[-max_waits:]
                new.append(inst)
            if changed:
                bb.instructions = new
    return nc


# ---------------------------------------------------------------------------
# device program
# ---------------------------------------------------------------------------

_NC_CACHE = {}


def _build_nc(split_waits=True):
    key = ("nc", split_waits)
    if key in _NC_CACHE:
        return _NC_CACHE[key]
    _apply_patches()
    import concourse.bass as bass
    import concourse.tile as tile
    from concourse import mybir
    from contextlib import ExitStack

    f32 = mybir.dt.float32
    bf16 = mybir.dt.bfloat16
    RELU = mybir.ActivationFunctionType.Relu
    EXP = mybir.ActivationFunctionType.Exp

    nc = bass.Bass()

    xf_d = nc.declare_dram_parameter("xf", [C, HP * WP], bf16, isOutput=False)
    xl_d = nc.declare_dram_parameter("xl", [C, LOCP * WP], bf16, isOutput=False)
    # pre-conv weights: taps (dr0|dr1) stacked on 128 partitions, dr2 separate
    wpre_d = nc.declare_dram_parameter("wpre", [2 * C, 3 * C], bf16, isOutput=False)
    wpre2_d = nc.declare_dram_parameter("wpre2", [C, 3 * C], bf16, isOutput=False)
    b1_d = nc.declare_dram_parameter("b1", [C, 1], f32, isOutput=False)
    wfin_d = nc.declare_dram_parameter("wfin", [C, 9 * C], bf16, isOutput=False)
    b2_d = nc.declare_dram_parameter("b2", [C, 1], f32, isOutput=False)
    wq_d = nc.declare_dram_parameter("wq", [C + 1, CQK], bf16, isOutput=False)
    wk_d = nc.declare_dram_parameter("wk", [C + 1, CQK], bf16, isOutput=False)
    wv_d = nc.declare_dram_parameter("wv", [C + 1, C + 1], bf16, isOutput=False)
    m2_d = nc.declare_dram_parameter("m2", [C, 2 * W], f32, isOutput=False)
    out_d = nc.declare_dram_parameter("out", [C, ROWS * W], f32, isOutput=True)

    taps9 = [(dr, ds) for dr in range(3) for ds in range(3)]

    with tile.TileContext(nc) as tc, ExitStack() as ctx:
        consts = ctx.enter_context(tc.tile_pool(name="consts", bufs=1))
        bigs = ctx.enter_context(tc.tile_pool(name="bigs", bufs=1))

        # --- constants ---
        wpre_sb = consts.tile([2 * C, 3 * C], bf16)
        wpre2_sb = consts.tile([C, 3 * C], bf16)
        wfin_sb = consts.tile([C, 9 * C], bf16)
        b1_sb = consts.tile([C, 1], f32)
        b2_sb = consts.tile([C, 1], f32)
        wq_sb = consts.tile([C + 1, CQK], bf16)
        wk_sb = consts.tile([C + 1, CQK], bf16)
        wv_sb = consts.tile([C + 1, C + 1], bf16)
        m2_sb = consts.tile([C, 2 * W], f32)
        dum_sb = consts.tile([1, 8], f32)
        nc.sync.dma_start(out=wpre_sb, in_=wpre_d[:])
        nc.sync.dma_start(out=wpre2_sb, in_=wpre2_d[:])
        nc.sync.dma_start(out=b1_sb, in_=b1_d[:])
        nc.sync.dma_start(out=wq_sb, in_=wq_d[:])
        nc.sync.dma_start(out=wk_sb, in_=wk_d[:])
        nc.sync.dma_start(out=wv_sb, in_=wv_d[:])
        nc.sync.dma_start(out=wfin_sb, in_=wfin_d[:])
        nc.sync.dma_start(out=b2_sb, in_=b2_d[:])
        nc.sync.dma_start(out=m2_sb, in_=m2_d[:])

        # --- big SBUF buffers ---
        xf_sb = bigs.tile([2 * C, HP * WP], bf16)
        xl_sb = bigs.tile([2 * C, LOCP * WP], bf16)
        ya_sb = bigs.tile([C + 1, N], bf16)       # y_aug (full sample)
        yla_sb = bigs.tile([C + 1, NLOC], bf16)   # y_aug (local window)
        ylf_sb = bigs.tile([C, NLOC], f32)        # y local fp32 (residual)
        k_sb = bigs.tile([80, N], bf16)           # k at partition offsets 0/32/64
        q_sb = bigs.tile([80, NLOC], bf16)
        vt_sb = bigs.tile([128, NJB * VB], bf16)
        of_sb = bigs.tile([C, NLOC], f32)
        ofp_sb = bigs.tile([C, LOCROWS * WP], bf16)
        out_sb = bigs.tile([C, ROWS * W], f32)

        nc.vector.memset(ya_sb[C:C + 1, :], 1.0)
        nc.vector.memset(yla_sb[C:C + 1, :], 1.0)
        nc.vector.memset(ofp_sb[:], 0.0)
        # exp table pre-load: tiny dummy activation early on the scalar queue
        nc.vector.memset(dum_sb[:], 0.0)
        nc.scalar.activation(out=dum_sb[:], in_=dum_sb[:], func=EXP)

        # --- input DMAs, 8-row chunked so compute starts early ---
        # local window (28 padded rows): chunks [0:8),[8:16),[16:24),[24:28)
        for r0, r1 in [(0, 8), (8, 16), (16, 24), (24, LOCP)]:
            nc.sync.dma_start(out=xl_sb[0:C, r0 * WP:r1 * WP],
                              in_=xl_d[:, r0 * WP:r1 * WP])
            s1 = min(r1, LOCP - 1)
            nc.sync.dma_start(out=xl_sb[C:2 * C, r0 * WP:s1 * WP],
                              in_=xl_d[:, (r0 + 1) * WP:(s1 + 1) * WP])
        # full frame (98 padded rows): chunks of 8 (last 10)
        fchunks = [(8 * i, 8 * i + 8) for i in range(11)] + [(88, HP)]
        for r0, r1 in fchunks:
            nc.sync.dma_start(out=xf_sb[0:C, r0 * WP:r1 * WP],
                              in_=xf_d[:, r0 * WP:r1 * WP])
            s1 = min(r1, HP - 1)
            nc.sync.dma_start(out=xf_sb[C:2 * C, r0 * WP:s1 * WP],
                              in_=xf_d[:, (r0 + 1) * WP:(s1 + 1) * WP])

        xf3 = xf_sb.rearrange("p (r c) -> p r c", c=WP)
        xl3 = xl_sb.rearrange("p (r c) -> p r c", c=WP)
        of3 = of_sb.rearrange("p (r c) -> p r c", c=W)
        m23 = m2_sb.rearrange("p (r c) -> p r c", c=W)
        ofp3 = ofp_sb.rearrange("p (r c) -> p r c", c=WP)

        def conv6(ps, x3, r, nr, stop_dr2):
            """6-matmul 3x3 conv chunk: rows r..r+nr of the padded frame."""
            for ds in range(3):
                nc.tensor.matmul(
                    ps[:, :nr * W],
                    wpre_sb[:, ds * C:(ds + 1) * C],
                    x3[:, r:r + nr, ds:ds + W],
                    start=(ds == 0), stop=False,
                )
            for ds in range(3):
                nc.tensor.matmul(
                    ps[:, :nr * W],
                    wpre2_sb[:, ds * C:(ds + 1) * C],
                    x3[0:C, r + 2:r + 2 + nr, ds:ds + W],
                    start=False, stop=(stop_dr2 and ds == 2),
                )

        # =================================================================
        # HEAD: local conv + q, then full conv / k / vt production.
        # PSUM: conv 2x2 banks, kq 2x1, vt 2x1 = 8 banks.
        # =================================================================
        with tc.tile_pool(name="head_ps", bufs=2, space="PSUM") as head_ps:
            # --- local window pre-conv -> yla (scalar act) + ylf (vector) ---
            for m, nr in [(0, 4), (4, 4), (8, 4), (12, 4), (16, 4), (20, 4),
                          (24, 2)]:
                ps = head_ps.tile([C, 4 * W], f32, tag="conv_ps")
                conv6(ps, xl3, m, nr, True)
                nc.scalar.activation(
                    out=yla_sb[0:C, m * W:(m + nr) * W],
                    in_=ps[:, :nr * W], func=RELU, bias=b1_sb[:, 0:1], scale=1.0,
                )
                nc.vector.tensor_scalar(
                    out=ylf_sb[:, m * W:(m + nr) * W], in0=ps[:, :nr * W],
                    scalar1=b1_sb[:, 0:1], scalar2=0.0,
                    op0=mybir.AluOpType.add, op1=mybir.AluOpType.max,
                )
            # --- q projection over the local window (5 chunks of 512) ---
            ioff = 0
            for sz in NI_SIZES:
                ps = head_ps.tile([CQK, 512], f32, tag="kq_ps")
                nc.tensor.matmul(ps[:, :sz], wq_sb[:], yla_sb[:, ioff:ioff + sz],
                                 start=True, stop=True)
                nc.scalar.copy(out=q_sb[0:CQK, ioff:ioff + sz], in_=ps[:, :sz])
                ioff += sz
            nc.sync.dma_start(out=q_sb[32:32 + CQK, :], in_=q_sb[0:CQK, :])
            nc.sync.dma_start(out=q_sb[64:64 + CQK, :], in_=q_sb[0:CQK, :])

            # --- full-frame pre-conv (24 chunks of 4 rows) + k + vt ---
            kc = 0      # next k chunk (512 cols) to emit
            for ch in range(24):
                ps = head_ps.tile([C, 4 * W], f32, tag="conv_ps")
                conv6(ps, xf3, ch * 4, 4, True)
                nc.scalar.activation(
                    out=ya_sb[0:C, ch * 4 * W:(ch + 1) * 4 * W],
                    in_=ps[:], func=RELU, bias=b1_sb[:, 0:1], scale=1.0,
                )
                # k projection: chunk covers cols [512*kc, 512*kc+512)
                while 512 * (kc + 1) <= (ch + 1) * 4 * W:
                    kps = head_ps.tile([CQK, 512], f32, tag="kq_ps")
                    nc.tensor.matmul(kps[:], wk_sb[:],
                                     ya_sb[:, kc * 512:(kc + 1) * 512],
                                     start=True, stop=True)
                    nc.vector.tensor_copy(out=k_sb[0:CQK, kc * 512:(kc + 1) * 512],
                                          in_=kps[:])
                    nc.sync.dma_start(
                        out=k_sb[32:32 + CQK, kc * 512:(kc + 1) * 512],
                        in_=k_sb[0:CQK, kc * 512:(kc + 1) * 512])
                    nc.sync.dma_start(
                        out=k_sb[64:64 + CQK, kc * 512:(kc + 1) * 512],
                        in_=k_sb[0:CQK, kc * 512:(kc + 1) * 512])
                    kc += 1
                # vt: 6 j-blocks per pair of conv chunks (768 cols = 6*128)
                if ch % 2 == 1:
                    vg = ch // 2
                    vps = head_ps.tile([128, 6 * VB], f32, tag="vt_ps")
                    for t in range(6):
                        jb = vg * 6 + t
                        nc.tensor.matmul(
                            vps[:, t * VB:(t + 1) * VB],
                            ya_sb[:, jb * JB:(jb + 1) * JB],
                            wv_sb[:], start=True, stop=True,
                        )
                    nc.vector.tensor_copy(
                        out=vt_sb[:, vg * 6 * VB:(vg + 1) * 6 * VB], in_=vps[:])

        # =================================================================
        # ATTENTION: software-pipelined exp stream.
        # PSUM: et 2x3 banks, acc 1, fin 1 = 8 banks.
        # =================================================================
        with tc.tile_pool(name="et_ps", bufs=2, space="PSUM") as et_ps, \
             tc.tile_pool(name="acc_ps", bufs=1, space="PSUM") as acc_ps, \
             tc.tile_pool(name="fin_ps", bufs=1, space="PSUM") as fin_ps, \
             tc.tile_pool(name="p_pool", bufs=3) as p_pool, \
             tc.tile_pool(name="ep_pool", bufs=2) as ep_pool, \
             tc.tile_pool(name="dram", bufs=2, space="DRAM") as dpool:

            IOFF = [0, 512, 1024, 1536, 2048]
            et_done = [None]
            et_tiles = {}
            p_tiles = {}
            acc_tiles = {}

            def emit_energy(G):
                ci, g = divmod(G, NJG)
                NI = NI_SIZES[ci]
                et = et_ps.tile([128, JG * 512], f32, tag="et", name="et")
                et_tiles[G] = et
                for t in range(JG):
                    jb = g * JG + t
                    nc.tensor.matmul(
                        et[:, t * 512:t * 512 + NI],
                        k_sb[32 * t:32 * t + CQK, jb * JB:(jb + 1) * JB],
                        q_sb[32 * t:32 * t + CQK, IOFF[ci]:IOFF[ci] + NI],
                        start=True, stop=True,
                        tile_position=(32 * t, 0),
                    )

            def emit_exp(G):
                ci, g = divmod(G, NJG)
                NI = NI_SIZES[ci]
                et = et_tiles.pop(G)
                et_done[0] = et
                p = p_pool.tile([128, JG * 512], bf16, tag="p", name="p")
                p_tiles[G] = p
                # single full-width ACT even for the 448 chunk: the gap
                # columns hold exp(stale PSUM) and are never read by PV.
                nc.scalar.activation(out=p[:], in_=et[:], func=EXP)

            def emit_pv(G):
                ci, g = divmod(G, NJG)
                NI = NI_SIZES[ci]
                if g == 0:
                    acc_tiles[ci] = acc_ps.tile([VB, 512], f32, tag="acc", name="acc")
                acc = acc_tiles[ci]
                p = p_tiles.pop(G)
                for t in range(JG):
                    jb = g * JG + t
                    nc.tensor.matmul(
                        acc[:, :NI],
                        vt_sb[:, jb * VB:(jb + 1) * VB],
                        p[:, t * 512:t * 512 + NI],
                        start=(g == 0 and t == 0),
                        stop=(g == NJG - 1 and t == JG - 1),
                    )

            def emit_epilogue(ci):
                """All vector/DMA: of = acc[0:64]*(gamma/s) + ylf, mask, pack."""
                NI = NI_SIZES[ci]
                ioff = IOFF[ci]
                acc = acc_tiles.pop(ci)
                acc_sb = ep_pool.tile([VB, 512], f32, tag="acc_sb", name="acc_sb")
                nc.vector.tensor_copy(out=acc_sb[:, :NI], in_=acc[:, :NI])
                # r = gamma / s  (gamma baked into the wv ones column)
                r = ep_pool.tile([1, 512], f32, tag="r", name="r")
                nc.vector.reciprocal(r[:, :NI], acc_sb[C:C + 1, :NI])
                rd = dpool.tile([1, 512], f32, tag="rd", name="rd")
                nc.sync.dma_start(out=rd[:, :NI], in_=r[:, :NI])
                rb = ep_pool.tile([C, 512], f32, tag="rb", name="rb")
                rd_bcast = bass.AP(tensor=rd.tensor, offset=rd.offset,
                                   ap=[[0, C]] + list(rd.ap[1:]))
                nc.sync.dma_start(out=rb[:, :NI], in_=rd_bcast[:, :NI])
                nc.vector.tensor_mul(of_sb[:, ioff:ioff + NI], acc_sb[0:C, :NI],
                                     rb[:, :NI])
                nc.vector.tensor_add(of_sb[:, ioff:ioff + NI],
                                     of_sb[:, ioff:ioff + NI],
                                     ylf_sb[:, ioff:ioff + NI])
                # mask halo rows that fall outside the image, then pack
                # completed rows into the padded layout for the final conv.
                r0 = ioff // W
                r1 = (ioff + NI) // W if ci < len(NI_SIZES) - 1 else LOCROWS
                if ci == 0:
                    nc.vector.tensor_mul(of3[:, 0:1, :], of3[:, 0:1, :],
                                         m23[:, 0:1, :])
                if ci == len(NI_SIZES) - 1:
                    nc.vector.tensor_mul(of3[:, LOCROWS - 1:LOCROWS, :],
                                         of3[:, LOCROWS - 1:LOCROWS, :],
                                         m23[:, 1:2, :])
                if r1 > r0:
                    nc.vector.tensor_copy(out=ofp3[:, r0:r1, 1:1 + W],
                                          in_=of3[:, r0:r1, :])

            fin_tiles = {}

            def emit_finconv_taps(fc, t0, t1, on_scalar):
                """Final 3x3 conv chunk fc (4 output rows), taps t0..t1-1."""
                if t0 == 0:
                    fin_tiles[fc] = fin_ps.tile([C, 4 * W], f32, tag="fin",
                                                name="fin")
                ps = fin_tiles[fc]
                for t in range(t0, t1):
                    dr, ds = taps9[t]
                    nc.tensor.matmul(
                        ps[:],
                        wfin_sb[:, t * C:(t + 1) * C],
                        ofp3[:, fc * 4 + dr:fc * 4 + dr + 4, ds:ds + W],
                        start=(t == 0), stop=(t == 8),
                    )
                if t1 < 9:
                    return
                del fin_tiles[fc]
                if on_scalar:
                    nc.scalar.activation(
                        out=out_sb[:, fc * 4 * W:(fc + 1) * 4 * W],
                        in_=ps[:], func=RELU, bias=b2_sb[:, 0:1], scale=1.0,
                    )
                else:
                    nc.vector.tensor_scalar(
                        out=out_sb[:, fc * 4 * W:(fc + 1) * 4 * W], in0=ps[:],
                        scalar1=b2_sb[:, 0:1], scalar2=0.0,
                        op0=mybir.AluOpType.add, op1=mybir.AluOpType.max,
                    )
                nc.sync.dma_start(out=out_d[:, fc * 4 * W:(fc + 1) * 4 * W],
                                  in_=out_sb[:, fc * 4 * W:(fc + 1) * 4 * W])

            # final-conv chunk fc reads ofp rows fc*4..fc*4+5, so it becomes
            # ready once the epilogue that packs row fc*4+5 completes
            # (~13us of vector work after its emission).  Emit each fin
            # chunk well after that, split 3 taps per iteration so its row
            # waits never block the PE queue for long.
            # fc needs of row fc*4+5: fc 0,1 -> epi1; 2 -> epi2; 3 -> epi3.
            # fin4's dr<=1 taps read only rows <=20 (epi3) and run in-stream;
            # its dr=2 taps and all of fin5 need epi4 and go in the tail.
            # One tap per iteration: a fin tap (~0.4us) fits the per-group PE
            # slack; 3-tap bursts were stalling the PE queue mid-stream.
            fin_taps = {}
            for fc, base in [(0, 57), (1, 67), (2, 91), (3, 103)]:
                for t in range(9):
                    fin_taps[base + t] = (fc, t, t + 1)
            for t in range(6):          # fin4 taps 0-5 (dr<=1) in-stream
                fin_taps[113 + t] = (4, t, t + 1)

            for it in range(NG):
                if it in fin_taps:
                    fc, t0, t1 = fin_taps[it]
                    emit_finconv_taps(fc, t0, t1, on_scalar=False)
                if it % NJG != 0 or it == 0:
                    emit_energy(it)
                emit_exp(it)
                emit_pv(it)
                if et_done[0] is not None:
                    # dependency-free filler into the consumed et slot keeps
                    # the PE dense so HAM stays at full clock
                    nc.tensor.matmul(et_done[0][0:VB, 0:512], vt_sb[:, 0:VB],
                                     vt_sb[:, 0:512], start=True, stop=True)
                    et_done[0] = None
                if it % NJG == NJG - 2 and it + 2 < NG:
                    # prefetch the next chunk's first energy group so the
                    # exp stream has no gap across the chunk boundary
                    emit_energy(it + 2)
                if it % NJG == NJG - 1:
                    emit_epilogue(it // NJG)

            # tail: finish fin4, then fin5 (scalar is free now)
            emit_finconv_taps(4, 6, 9, on_scalar=True)
            for t0 in range(0, 9, 3):
                emit_finconv_taps(5, t0, t0 + 3, on_scalar=True)


    if split_waits:
        _split_excess_waits(nc)
    _NC_CACHE[key] = nc
    return nc


# ---------------------------------------------------------------------------
# host-side prep + launch
# ---------------------------------------------------------------------------

def _prep_in_maps(x, w_pre, bn1_g, bn1_b, bn1_m, bn1_v, wq, bq, wk, bk, wv, bv,
                  w_fin, bn2_g, bn2_b, bn2_m, bn2_v, gamma):
    x = np.asarray(x, np.float32)
    inv1 = 1.0 / np.sqrt(np.asarray(bn1_v, np.float32) + EPS)
    s1 = np.asarray(bn1_g, np.float32) * inv1
    wpre_f = np.asarray(w_pre, np.float32) * s1[:, None, None, None]
    b1f = np.asarray(bn1_b, np.float32) - np.asarray(bn1_m, np.float32) * s1
    inv2 = 1.0 / np.sqrt(np.asarray(bn2_v, np.float32) + EPS)
    s2 = np.asarray(bn2_g, np.float32) * inv2
    wfin_f = np.asarray(w_fin, np.float32) * s2[:, None, None, None]
    b2f = np.asarray(bn2_b, np.float32) - np.asarray(bn2_m, np.float32) * s2

    # pre-conv weights, 2-row-packed: [dr0|dr1] on 128 partitions, dr2 alone
    # lhsT layout per (dr, ds) tap: [cin, cout]
    wt = wpre_f.transpose(1, 2, 3, 0)        # [cin, dr, ds, cout]
    wpre_pack = np.concatenate([wt[:, 0], wt[:, 1]], axis=0)  # [128, 3, 64]
    wpre_pack = wpre_pack.reshape(2 * C, 3 * C).astype(BF16)
    wpre2 = wt[:, 2].reshape(C, 3 * C).astype(BF16)
    # final conv, 9-tap layout [cin, tap, cout]
    wfin_t = np.ascontiguousarray(
        wfin_f.transpose(1, 2, 3, 0).reshape(C, 9 * C)).astype(BF16)

    gma = float(np.asarray(gamma, np.float32).reshape(-1)[0])
    wq2 = np.asarray(wq, np.float32).reshape(CQK, C)
    wk2 = np.asarray(wk, np.float32).reshape(CQK, C)
    wv2 = np.asarray(wv, np.float32).reshape(C, C)
    wq_aug = np.concatenate([wq2.T, np.asarray(bq, np.float32)[None, :]], 0).astype(BF16)
    wk_aug = np.concatenate([wk2.T, np.asarray(bk, np.float32)[None, :]], 0).astype(BF16)
    wv_aug = np.zeros((C + 1, C + 1), np.float32)
    wv_aug[0:C, 0:C] = wv2.T
    wv_aug[C, 0:C] = np.asarray(bv, np.float32)
    # gamma folded into the softmax-denominator column: acc row 64 = s/gamma,
    # so reciprocal directly yields gamma/s.
    wv_aug[C, C] = 1.0 / gma
    wv_aug = wv_aug.astype(BF16)

    b1f = b1f.reshape(C, 1)
    b2f = b2f.reshape(C, 1)

    xpad = np.zeros((B, C, HP, WP), np.float32)
    xpad[:, :, 1:1 + H, 1:1 + W] = x
    xpad_bf = xpad.astype(BF16)

    in_maps = []
    for core in range(8):
        b, qc = divmod(core, QCH)
        xf = xpad_bf[b].reshape(C, HP * WP)
        # local window: image rows [24q-2, 24q+26) = padded rows [24q-1, 24q+27)
        xl = np.zeros((C, LOCP, WP), np.float32)
        pr0 = ROWS * qc - 1
        lo = max(0, -pr0)
        hi = min(LOCP, HP - pr0)
        xl[:, lo:hi, :] = xpad[b, :, pr0 + lo:pr0 + hi, :]
        xl = xl.reshape(C, LOCP * WP).astype(BF16)
        m2 = np.ones((C, 2 * W), np.float32)
        if qc == 0:
            m2[:, 0:W] = 0.0
        if qc == QCH - 1:
            m2[:, W:2 * W] = 0.0
        in_maps.append({
            "xf": xf, "xl": xl, "wpre": wpre_pack, "wpre2": wpre2, "b1": b1f,
            "wfin": wfin_t, "b2": b2f, "wq": wq_aug, "wk": wk_aug,
            "wv": wv_aug, "m2": m2,
        })
    return in_maps


def kernel(**inputs):
    from concourse.bass_utils import run_bass_kernel_spmd

    nc = _build_nc()
    in_maps = _prep_in_maps(**inputs)
    res = run_bass_kernel_spmd(nc, in_maps, list(range(8)))
    out = np.zeros((B, C, H, W), np.float32)
    for core in range(8):
        b, qc = divmod(core, QCH)
        out[b, :, ROWS * qc:ROWS * (qc + 1), :] = \
            res.results[core]["out"].reshape(C, ROWS, W)
    return out


# revision 22
# speedup vs baseline: 1.0924x; 1.0924x over previous
"""Trainium2 Bass kernel for nn_AttentionLayer (pre-conv + self-attention + final conv).

Sharding: 8 cores = 2 samples x 4 query-row chunks. Each core computes the
full pre-conv y for its sample (k/v need all N=9216 positions), attention for
its 26-row query window (24 own rows + 1 halo row each side for the final
3x3 conv), and the final conv for its 24 output rows.

Perf structure (v2): the kernel is a producer/consumer pipeline built around
the scalar-engine exp stream (the hard bottleneck: ~23M softmax elements at
1 elem/lane/cycle). The head streams input DMAs in 8-row chunks so the
pre-conv starts ~1us in, produces y/k/q/vt with PSUM at full width, and
splits PSUM evacuations between the scalar and vector engines. The attention
loop is software-pipelined one group ahead (emit order per group G:
PV(G-2), Energy(G), exp(G-1)) so the scalar engine never waits for the
tensor engine. Per-chunk epilogues (1/s broadcast, residual add, row pack)
run entirely on vector+DMA, and final-conv chunks are interleaved into the
stream as their input rows complete.
"""

import os
import hashlib
import shutil

import numpy as np
import ml_dtypes

BF16 = ml_dtypes.bfloat16
EPS = 1e-5

B, C, CQK, H, W = 2, 64, 16, 96, 96
N = H * W                       # 9216
QCH = 4                         # query chunks per sample
ROWS = H // QCH                 # 24 rows per core
LOCROWS = ROWS + 2              # 26 (with halo)
NLOC = LOCROWS * W              # 2496
HP, WP = H + 2, W + 2           # 98x98 padded frame
LOCP = LOCROWS + 2              # 28 padded local rows
NI_SIZES = [512, 512, 512, 512, 448]   # i-chunks over NLOC
JB = 128                        # j-block height
NJB = N // JB                   # 72
JG = 3                          # j-blocks per exp group (3-way tile_position)
NJG = NJB // JG                 # 24 groups per i-chunk
NG = NJG * len(NI_SIZES)        # 120 total groups
VB = C + 1                      # 65


# ---------------------------------------------------------------------------
# framework patches (self-contained)
# ---------------------------------------------------------------------------

def _apply_patches():
    import concourse.tile as tile
    import concourse.bass_utils as bu
    import concourse.bass2jax as b2j
    from concourse import mybir

    # 1) walrus in this env rejects >1-2 sync waits on the final Drain
    #    (CTRL_NO_STRUCT): split waits into single-wait nops.
    def _drain_and_barrier_split(self, tick_clock, wait_clock):
        nc = self.nc
        probe = nc.sync.nop()
        wait_clock.add_sem_waits(
            probe.ins, tile.ScopedClock({None: tick_clock.global_clock})
        )
        waits = list(probe.ins.sync_info.on_wait) if probe.ins.sync_info else []
        if probe.ins.sync_info is not None:
            probe.ins.sync_info.on_wait = []
        for w in waits[:-1]:
            nop = nc.sync.nop()
            if nop.ins.sync_info is None:
                nop.ins.sync_info = mybir.SyncInfo(on_wait=[w], on_update=[])
            else:
                nop.ins.sync_info.on_wait.append(w)
        drain_inst = nc.sync.drain()
        if waits:
            if drain_inst.ins.sync_info is None:
                drain_inst.ins.sync_info = mybir.SyncInfo(
                    on_wait=[waits[-1]], on_update=[]
                )
            else:
                drain_inst.ins.sync_info.on_wait.append(waits[-1])
        nc.all_engine_barrier()
        assert self.sems is not None
        popped = nc._tile_sem_poison_stack.pop()
        assert popped is self._sem_poison
        nc.clear_and_free_semaphores(list(self.sems.allocated().values()))
        nc.all_engine_barrier()

    tile.TileContext._drain_and_barrier = _drain_and_barrier_split

    # 2) NEFF disk cache keyed by BIR hash (compile is deterministic).
    cache_dir = os.path.join(os.path.dirname(os.path.abspath(__file__)),
                             ".neff_cache")
    try:
        os.makedirs(cache_dir, exist_ok=True)
    except OSError:
        cache_dir = None
    _orig_compile = bu.compile_bir_kernel

    def cached_compile(bir_json, tmpdir, neff_name="file.neff"):
        if cache_dir is None:
            return _orig_compile(bir_json, tmpdir, neff_name)
        h = hashlib.sha256(bir_json).hexdigest()[:24]
        cpath = os.path.join(cache_dir, f"{h}.neff")
        out = os.path.join(tmpdir, neff_name)
        if os.path.exists(cpath):
            shutil.copyfile(cpath, out)
            return out
        r = _orig_compile(bir_json, tmpdir, neff_name)
        try:
            shutil.copyfile(r, cpath)
        except OSError:
            pass
        return r

    bu.compile_bir_kernel = cached_compile
    b2j.compile_bir_kernel = cached_compile


def _split_excess_waits(nc, max_waits=1):
    """walrus in this env allows only a couple of sync-wait slots per
    instruction; move excess waits onto preceding same-engine NOPs."""
    from concourse import mybir
    idx = 0
    for f in nc.m.functions:
        for bb in f.blocks:
            new = []
            changed = False
            for inst in bb.instructions:
                si = inst.sync_info
                waits = list(si.on_wait) if si is not None and si.on_wait else []
                if len(waits) > max_waits:
                    changed = True
                    for w in waits[:-max_waits]:
                        idx += 1
                        nop = mybir.InstNoOp(name=f"wsplit_{idx}", ins=[], outs=[])
                        nop.engine = inst.engine
                        nop.sync_info = mybir.SyncInfo(on_wait=[w], on_update=[])
                        new.append(nop)
                    si.on_wait = waits[-max_waits:]
                new.append(inst)
            if changed:
                bb.instructions = new
    return nc


# ---------------------------------------------------------------------------
# device program
# ---------------------------------------------------------------------------

_NC_CACHE = {}


def _build_nc(split_waits=True):
    key = ("nc", split_waits)
    if key in _NC_CACHE:
        return _NC_CACHE[key]
    _apply_patches()
    import concourse.bass as bass
    import concourse.tile as tile
    from concourse import mybir
    from contextlib import ExitStack

    f32 = mybir.dt.float32
    bf16 = mybir.dt.bfloat16
    RELU = mybir.ActivationFunctionType.Relu
    EXP = mybir.ActivationFunctionType.Exp

    nc = bass.Bass()

    xf_d = nc.declare_dram_parameter("xf", [C, HP * WP], bf16, isOutput=False)
    xl_d = nc.declare_dram_parameter("xl", [C, LOCP * WP], bf16, isOutput=False)
    # pre-conv weights: taps (dr0|dr1) stacked on 128 partitions, dr2 separate
    wpre_d = nc.declare_dram_parameter("wpre", [2 * C, 3 * C], bf16, isOutput=False)
    wpre2_d = nc.declare_dram_parameter("wpre2", [C, 3 * C], bf16, isOutput=False)
    b1_d = nc.declare_dram_parameter("b1", [C, 1], f32, isOutput=False)
    wfin_d = nc.declare_dram_parameter("wfin", [C, 9 * C], bf16, isOutput=False)
    b2_d = nc.declare_dram_parameter("b2", [C, 1], f32, isOutput=False)
    wq_d = nc.declare_dram_parameter("wq", [C + 1, CQK], bf16, isOutput=False)
    wk_d = nc.declare_dram_parameter("wk", [C + 1, CQK], bf16, isOutput=False)
    wv_d = nc.declare_dram_parameter("wv", [C + 1, C + 1], bf16, isOutput=False)
    m2_d = nc.declare_dram_parameter("m2", [C, 2 * W], f32, isOutput=False)
    out_d = nc.declare_dram_parameter("out", [C, ROWS * W], f32, isOutput=True)

    taps9 = [(dr, ds) for dr in range(3) for ds in range(3)]

    with tile.TileContext(nc) as tc, ExitStack() as ctx:
        consts = ctx.enter_context(tc.tile_pool(name="consts", bufs=1))
        bigs = ctx.enter_context(tc.tile_pool(name="bigs", bufs=1))

        # --- constants ---
        wpre_sb = consts.tile([2 * C, 3 * C], bf16)
        wpre2_sb = consts.tile([C, 3 * C], bf16)
        wfin_sb = consts.tile([C, 9 * C], bf16)
        b1_sb = consts.tile([C, 1], f32)
        b2_sb = consts.tile([C, 1], f32)
        wq_sb = consts.tile([C + 1, CQK], bf16)
        wk_sb = consts.tile([C + 1, CQK], bf16)
        wv_sb = consts.tile([C + 1, C + 1], bf16)
        m2_sb = consts.tile([C, 2 * W], f32)
        dum_sb = consts.tile([1, 8], f32)
        nc.sync.dma_start(out=wpre_sb, in_=wpre_d[:])
        nc.sync.dma_start(out=wpre2_sb, in_=wpre2_d[:])
        nc.sync.dma_start(out=b1_sb, in_=b1_d[:])
        nc.sync.dma_start(out=wq_sb, in_=wq_d[:])
        nc.sync.dma_start(out=wk_sb, in_=wk_d[:])
        nc.sync.dma_start(out=wv_sb, in_=wv_d[:])
        nc.sync.dma_start(out=wfin_sb, in_=wfin_d[:])
        nc.sync.dma_start(out=b2_sb, in_=b2_d[:])
        nc.sync.dma_start(out=m2_sb, in_=m2_d[:])

        # --- big SBUF buffers ---
        xf_sb = bigs.tile([2 * C, HP * WP], bf16)
        xl_sb = bigs.tile([2 * C, LOCP * WP], bf16)
        ya_sb = bigs.tile([C + 1, N], bf16)       # y_aug (full sample)
        yla_sb = bigs.tile([C + 1, NLOC], bf16)   # y_aug (local window)
        ylf_sb = bigs.tile([C, NLOC], f32)        # y local fp32 (residual)
        k_sb = bigs.tile([80, N], bf16)           # k at partition offsets 0/32/64
        q_sb = bigs.tile([80, NLOC], bf16)
        vt_sb = bigs.tile([128, NJB * VB], bf16)
        of_sb = bigs.tile([C, NLOC], f32)
        ofp_sb = bigs.tile([C, LOCROWS * WP], bf16)
        out_sb = bigs.tile([C, ROWS * W], f32)

        nc.vector.memset(ya_sb[C:C + 1, :], 1.0)
        nc.vector.memset(yla_sb[C:C + 1, :], 1.0)
        nc.vector.memset(ofp_sb[:], 0.0)
        # exp table pre-load: tiny dummy activation early on the scalar queue
        nc.vector.memset(dum_sb[:], 0.0)
        nc.scalar.activation(out=dum_sb[:], in_=dum_sb[:], func=EXP)

        # --- input DMAs, 8-row chunked so compute starts early ---
        # local window (28 padded rows): chunks [0:8),[8:16),[16:24),[24:28)
        for r0, r1 in [(0, 8), (8, 16), (16, 24), (24, LOCP)]:
            nc.sync.dma_start(out=xl_sb[0:C, r0 * WP:r1 * WP],
                              in_=xl_d[:, r0 * WP:r1 * WP])
            s1 = min(r1, LOCP - 1)
            nc.sync.dma_start(out=xl_sb[C:2 * C, r0 * WP:s1 * WP],
                              in_=xl_d[:, (r0 + 1) * WP:(s1 + 1) * WP])
        # full frame (98 padded rows): chunks of 8 (last 10)
        fchunks = [(8 * i, 8 * i + 8) for i in range(11)] + [(88, HP)]
        for r0, r1 in fchunks:
            nc.sync.dma_start(out=xf_sb[0:C, r0 * WP:r1 * WP],
                              in_=xf_d[:, r0 * WP:r1 * WP])
            s1 = min(r1, HP - 1)
            nc.sync.dma_start(out=xf_sb[C:2 * C, r0 * WP:s1 * WP],
                              in_=xf_d[:, (r0 + 1) * WP:(s1 + 1) * WP])

        xf3 = xf_sb.rearrange("p (r c) -> p r c", c=WP)
        xl3 = xl_sb.rearrange("p (r c) -> p r c", c=WP)
        of3 = of_sb.rearrange("p (r c) -> p r c", c=W)
        m23 = m2_sb.rearrange("p (r c) -> p r c", c=W)
        ofp3 = ofp_sb.rearrange("p (r c) -> p r c", c=WP)

        def conv6(ps, x3, r, nr, stop_dr2):
            """6-matmul 3x3 conv chunk: rows r..r+nr of the padded frame."""
            for ds in range(3):
                nc.tensor.matmul(
                    ps[:, :nr * W],
                    wpre_sb[:, ds * C:(ds + 1) * C],
                    x3[:, r:r + nr, ds:ds + W],
                    start=(ds == 0), stop=False,
                )
            for ds in range(3):
                nc.tensor.matmul(
                    ps[:, :nr * W],
                    wpre2_sb[:, ds * C:(ds + 1) * C],
                    x3[0:C, r + 2:r + 2 + nr, ds:ds + W],
                    start=False, stop=(stop_dr2 and ds == 2),
                )

        # =================================================================
        # HEAD: local conv + q, then full conv / k / vt production.
        # PSUM: conv 2x2 banks, kq 2x1, vt 2x1 = 8 banks.
        # =================================================================
        with tc.tile_pool(name="head_ps", bufs=2, space="PSUM") as head_ps:
            # --- local window pre-conv -> yla (scalar act) + ylf (vector) ---
            for m, nr in [(0, 4), (4, 4), (8, 4), (12, 4), (16, 4), (20, 4),
                          (24, 2)]:
                ps = head_ps.tile([C, 4 * W], f32, tag="conv_ps")
                conv6(ps, xl3, m, nr, True)
                nc.scalar.activation(
                    out=yla_sb[0:C, m * W:(m + nr) * W],
                    in_=ps[:, :nr * W], func=RELU, bias=b1_sb[:, 0:1], scale=1.0,
                )
                nc.vector.tensor_scalar(
                    out=ylf_sb[:, m * W:(m + nr) * W], in0=ps[:, :nr * W],
                    scalar1=b1_sb[:, 0:1], scalar2=0.0,
                    op0=mybir.AluOpType.add, op1=mybir.AluOpType.max,
                )
            # --- q projection over the local window (5 chunks of 512) ---
            ioff = 0
            for sz in NI_SIZES:
                ps = head_ps.tile([CQK, 512], f32, tag="kq_ps")
                nc.tensor.matmul(ps[:, :sz], wq_sb[:], yla_sb[:, ioff:ioff + sz],
                                 start=True, stop=True)
                nc.scalar.copy(out=q_sb[0:CQK, ioff:ioff + sz], in_=ps[:, :sz])
                ioff += sz
            nc.sync.dma_start(out=q_sb[32:32 + CQK, :], in_=q_sb[0:CQK, :])
            nc.sync.dma_start(out=q_sb[64:64 + CQK, :], in_=q_sb[0:CQK, :])

            # --- full-frame pre-conv (24 chunks of 4 rows) + k + vt ---
            kc = 0      # next k chunk (512 cols) to emit
            for ch in range(24):
                ps = head_ps.tile([C, 4 * W], f32, tag="conv_ps")
                conv6(ps, xf3, ch * 4, 4, True)
                nc.scalar.activation(
                    out=ya_sb[0:C, ch * 4 * W:(ch + 1) * 4 * W],
                    in_=ps[:], func=RELU, bias=b1_sb[:, 0:1], scale=1.0,
                )
                # k projection: chunk covers cols [512*kc, 512*kc+512)
                while 512 * (kc + 1) <= (ch + 1) * 4 * W:
                    kps = head_ps.tile([CQK, 512], f32, tag="kq_ps")
                    nc.tensor.matmul(kps[:], wk_sb[:],
                                     ya_sb[:, kc * 512:(kc + 1) * 512],
                                     start=True, stop=True)
                    nc.vector.tensor_copy(out=k_sb[0:CQK, kc * 512:(kc + 1) * 512],
                                          in_=kps[:])
                    nc.sync.dma_start(
                        out=k_sb[32:32 + CQK, kc * 512:(kc + 1) * 512],
                        in_=k_sb[0:CQK, kc * 512:(kc + 1) * 512])
                    nc.sync.dma_start(
                        out=k_sb[64:64 + CQK, kc * 512:(kc + 1) * 512],
                        in_=k_sb[0:CQK, kc * 512:(kc + 1) * 512])
                    kc += 1
                # vt: 6 j-blocks per pair of conv chunks (768 cols = 6*128)
                if ch % 2 == 1:
                    vg = ch // 2
                    vps = head_ps.tile([128, 6 * VB], f32, tag="vt_ps")
                    for t in range(6):
                        jb = vg * 6 + t
                        nc.tensor.matmul(
                            vps[:, t * VB:(t + 1) * VB],
                            ya_sb[:, jb * JB:(jb + 1) * JB],
                            wv_sb[:], start=True, stop=True,
                        )
                    nc.vector.tensor_copy(
                        out=vt_sb[:, vg * 6 * VB:(vg + 1) * 6 * VB], in_=vps[:])

        # =================================================================
        # ATTENTION: software-pipelined exp stream.
        # PSUM: et 2x3 banks, acc 1, fin 1 = 8 banks.
        # =================================================================
        with tc.tile_pool(name="et_ps", bufs=2, space="PSUM") as et_ps, \
             tc.tile_pool(name="acc_ps", bufs=1, space="PSUM") as acc_ps, \
             tc.tile_pool(name="fin_ps", bufs=1, space="PSUM") as fin_ps, \
             tc.tile_pool(name="p_pool", bufs=3) as p_pool, \
             tc.tile_pool(name="ep_pool", bufs=2) as ep_pool, \
             tc.tile_pool(name="dram", bufs=2, space="DRAM") as dpool:

            IOFF = [0, 512, 1024, 1536, 2048]
            et_tiles = {}
            p_tiles = {}
            acc_tiles = {}

            def emit_energy(G):
                ci, g = divmod(G, NJG)
                NI = NI_SIZES[ci]
                et = et_ps.tile([128, JG * 512], f32, tag="et", name="et")
                et_tiles[G] = et
                for t in range(JG):
                    jb = g * JG + t
                    nc.tensor.matmul(
                        et[:, t * 512:t * 512 + NI],
                        k_sb[32 * t:32 * t + CQK, jb * JB:(jb + 1) * JB],
                        q_sb[32 * t:32 * t + CQK, IOFF[ci]:IOFF[ci] + NI],
                        start=True, stop=True,
                        tile_position=(32 * t, 0),
                    )

            def emit_exp(G):
                ci, g = divmod(G, NJG)
                NI = NI_SIZES[ci]
                et = et_tiles.pop(G)
                p = p_pool.tile([128, JG * 512], bf16, tag="p", name="p")
                p_tiles[G] = p
                # single full-width ACT even for the 448 chunk: the gap
                # columns hold exp(stale PSUM) and are never read by PV.
                nc.scalar.activation(out=p[:], in_=et[:], func=EXP)

            def emit_pv(G):
                ci, g = divmod(G, NJG)
                NI = NI_SIZES[ci]
                if g == 0:
                    acc_tiles[ci] = acc_ps.tile([VB, 512], f32, tag="acc", name="acc")
                acc = acc_tiles[ci]
                p = p_tiles.pop(G)
                for t in range(JG):
                    jb = g * JG + t
                    nc.tensor.matmul(
                        acc[:, :NI],
                        vt_sb[:, jb * VB:(jb + 1) * VB],
                        p[:, t * 512:t * 512 + NI],
                        start=(g == 0 and t == 0),
                        stop=(g == NJG - 1 and t == JG - 1),
                    )

            def emit_epilogue(ci):
                """All vector/DMA: of = acc[0:64]*(gamma/s) + ylf, mask, pack."""
                NI = NI_SIZES[ci]
                ioff = IOFF[ci]
                acc = acc_tiles.pop(ci)
                acc_sb = ep_pool.tile([VB, 512], f32, tag="acc_sb", name="acc_sb")
                nc.vector.tensor_copy(out=acc_sb[:, :NI], in_=acc[:, :NI])
                # r = gamma / s  (gamma baked into the wv ones column)
                r = ep_pool.tile([1, 512], f32, tag="r", name="r")
                nc.vector.reciprocal(r[:, :NI], acc_sb[C:C + 1, :NI])
                rd = dpool.tile([1, 512], f32, tag="rd", name="rd")
                nc.sync.dma_start(out=rd[:, :NI], in_=r[:, :NI])
                rb = ep_pool.tile([C, 512], f32, tag="rb", name="rb")
                rd_bcast = bass.AP(tensor=rd.tensor, offset=rd.offset,
                                   ap=[[0, C]] + list(rd.ap[1:]))
                nc.sync.dma_start(out=rb[:, :NI], in_=rd_bcast[:, :NI])
                nc.vector.tensor_mul(of_sb[:, ioff:ioff + NI], acc_sb[0:C, :NI],
                                     rb[:, :NI])
                nc.vector.tensor_add(of_sb[:, ioff:ioff + NI],
                                     of_sb[:, ioff:ioff + NI],
                                     ylf_sb[:, ioff:ioff + NI])
                # mask halo rows that fall outside the image, then pack
                # completed rows into the padded layout for the final conv.
                r0 = ioff // W
                r1 = (ioff + NI) // W if ci < len(NI_SIZES) - 1 else LOCROWS
                if ci == 0:
                    nc.vector.tensor_mul(of3[:, 0:1, :], of3[:, 0:1, :],
                                         m23[:, 0:1, :])
                if ci == len(NI_SIZES) - 1:
                    nc.vector.tensor_mul(of3[:, LOCROWS - 1:LOCROWS, :],
                                         of3[:, LOCROWS - 1:LOCROWS, :],
                                         m23[:, 1:2, :])
                if r1 > r0:
                    nc.vector.tensor_copy(out=ofp3[:, r0:r1, 1:1 + W],
                                          in_=of3[:, r0:r1, :])

            fin_tiles = {}

            def emit_finconv_taps(fc, t0, t1, on_scalar):
                """Final 3x3 conv chunk fc (4 output rows), taps t0..t1-1."""
                if t0 == 0:
                    fin_tiles[fc] = fin_ps.tile([C, 4 * W], f32, tag="fin",
                                                name="fin")
                ps = fin_tiles[fc]
                for t in range(t0, t1):
                    dr, ds = taps9[t]
                    nc.tensor.matmul(
                        ps[:],
                        wfin_sb[:, t * C:(t + 1) * C],
                        ofp3[:, fc * 4 + dr:fc * 4 + dr + 4, ds:ds + W],
                        start=(t == 0), stop=(t == 8),
                    )
                if t1 < 9:
                    return
                del fin_tiles[fc]
                if on_scalar:
                    nc.scalar.activation(
                        out=out_sb[:, fc * 4 * W:(fc + 1) * 4 * W],
                        in_=ps[:], func=RELU, bias=b2_sb[:, 0:1], scale=1.0,
                    )
                else:
                    nc.vector.tensor_scalar(
                        out=out_sb[:, fc * 4 * W:(fc + 1) * 4 * W], in0=ps[:],
                        scalar1=b2_sb[:, 0:1], scalar2=0.0,
                        op0=mybir.AluOpType.add, op1=mybir.AluOpType.max,
                    )
                nc.sync.dma_start(out=out_d[:, fc * 4 * W:(fc + 1) * 4 * W],
                                  in_=out_sb[:, fc * 4 * W:(fc + 1) * 4 * W])

            # final-conv chunk fc reads ofp rows fc*4..fc*4+5, so it becomes
            # ready once the epilogue that packs row fc*4+5 completes
            # (~13us of vector work after its emission).  Emit each fin
            # chunk well after that, split 3 taps per iteration so its row
            # waits never block the PE queue for long.
            # fc needs of row fc*4+5: fc 0,1 -> epi1; 2 -> epi2; 3 -> epi3.
            # fin4's dr<=1 taps read only rows <=20 (epi3) and run in-stream;
            # its dr=2 taps and all of fin5 need epi4 and go in the tail.
            # One tap per iteration: a fin tap (~0.4us) fits the per-group PE
            # slack; 3-tap bursts were stalling the PE queue mid-stream.
            fin_taps = {}
            for fc, base in [(0, 57), (1, 67), (2, 91), (3, 103)]:
                for t in range(9):
                    fin_taps[base + t] = (fc, t, t + 1)
            for t in range(6):          # fin4 taps 0-5 (dr<=1) in-stream
                fin_taps[113 + t] = (4, t, t + 1)

            for it in range(NG):
                if it in fin_taps:
                    fc, t0, t1 = fin_taps[it]
                    emit_finconv_taps(fc, t0, t1, on_scalar=False)
                if it % NJG != 0 or it == 0:
                    emit_energy(it)
                emit_exp(it)
                emit_pv(it)
                if it % NJG == NJG - 2 and it + 2 < NG:
                    # prefetch the next chunk's first energy group so the
                    # exp stream has no gap across the chunk boundary
                    emit_energy(it + 2)
                if it % NJG == NJG - 1:
                    emit_epilogue(it // NJG)

            # tail: finish fin4, then fin5 (scalar is free now)
            emit_finconv_taps(4, 6, 9, on_scalar=True)
            for t0 in range(0, 9, 3):
                emit_finconv_taps(5, t0, t0 + 3, on_scalar=True)


    if split_waits:
        _split_excess_waits(nc)
    _NC_CACHE[key] = nc
    return nc


# ---------------------------------------------------------------------------
# host-side prep + launch
# ---------------------------------------------------------------------------

def _prep_in_maps(x, w_pre, bn1_g, bn1_b, bn1_m, bn1_v, wq, bq, wk, bk, wv, bv,
                  w_fin, bn2_g, bn2_b, bn2_m, bn2_v, gamma):
    x = np.asarray(x, np.float32)
    inv1 = 1.0 / np.sqrt(np.asarray(bn1_v, np.float32) + EPS)
    s1 = np.asarray(bn1_g, np.float32) * inv1
    wpre_f = np.asarray(w_pre, np.float32) * s1[:, None, None, None]
    b1f = np.asarray(bn1_b, np.float32) - np.asarray(bn1_m, np.float32) * s1
    inv2 = 1.0 / np.sqrt(np.asarray(bn2_v, np.float32) + EPS)
    s2 = np.asarray(bn2_g, np.float32) * inv2
    wfin_f = np.asarray(w_fin, np.float32) * s2[:, None, None, None]
    b2f = np.asarray(bn2_b, np.float32) - np.asarray(bn2_m, np.float32) * s2

    # pre-conv weights, 2-row-packed: [dr0|dr1] on 128 partitions, dr2 alone
    # lhsT layout per (dr, ds) tap: [cin, cout]
    wt = wpre_f.transpose(1, 2, 3, 0)        # [cin, dr, ds, cout]
    wpre_pack = np.concatenate([wt[:, 0], wt[:, 1]], axis=0)  # [128, 3, 64]
    wpre_pack = wpre_pack.reshape(2 * C, 3 * C).astype(BF16)
    wpre2 = wt[:, 2].reshape(C, 3 * C).astype(BF16)
    # final conv, 9-tap layout [cin, tap, cout]
    wfin_t = np.ascontiguousarray(
        wfin_f.transpose(1, 2, 3, 0).reshape(C, 9 * C)).astype(BF16)

    gma = float(np.asarray(gamma, np.float32).reshape(-1)[0])
    wq2 = np.asarray(wq, np.float32).reshape(CQK, C)
    wk2 = np.asarray(wk, np.float32).reshape(CQK, C)
    wv2 = np.asarray(wv, np.float32).reshape(C, C)
    wq_aug = np.concatenate([wq2.T, np.asarray(bq, np.float32)[None, :]], 0).astype(BF16)
    wk_aug = np.concatenate([wk2.T, np.asarray(bk, np.float32)[None, :]], 0).astype(BF16)
    wv_aug = np.zeros((C + 1, C + 1), np.float32)
    wv_aug[0:C, 0:C] = wv2.T
    wv_aug[C, 0:C] = np.asarray(bv, np.float32)
    # gamma folded into the softmax-denominator column: acc row 64 = s/gamma,
    # so reciprocal directly yields gamma/s.
    wv_aug[C, C] = 1.0 / gma
    wv_aug = wv_aug.astype(BF16)

    b1f = b1f.reshape(C, 1)
    b2f = b2f.reshape(C, 1)

    xpad = np.zeros((B, C, HP, WP), np.float32)
    xpad[:, :, 1:1 + H, 1:1 + W] = x
    xpad_bf = xpad.astype(BF16)

    in_maps = []
    for core in range(8):
        b, qc = divmod(core, QCH)
        xf = xpad_bf[b].reshape(C, HP * WP)
        # local window: image rows [24q-2, 24q+26) = padded rows [24q-1, 24q+27)
        xl = np.zeros((C, LOCP, WP), np.float32)
        pr0 = ROWS * qc - 1
        lo = max(0, -pr0)
        hi = min(LOCP, HP - pr0)
        xl[:, lo:hi, :] = xpad[b, :, pr0 + lo:pr0 + hi, :]
        xl = xl.reshape(C, LOCP * WP).astype(BF16)
        m2 = np.ones((C, 2 * W), np.float32)
        if qc == 0:
            m2[:, 0:W] = 0.0
        if qc == QCH - 1:
            m2[:, W:2 * W] = 0.0
        in_maps.append({
            "xf": xf, "xl": xl, "wpre": wpre_pack, "wpre2": wpre2, "b1": b1f,
            "wfin": wfin_t, "b2": b2f, "wq": wq_aug, "wk": wk_aug,
            "wv": wv_aug, "m2": m2,
        })
    return in_maps


def kernel(**inputs):
    from concourse.bass_utils import run_bass_kernel_spmd

    nc = _build_nc()
    in_maps = _prep_in_maps(**inputs)
    res = run_bass_kernel_spmd(nc, in_maps, list(range(8)))
    out = np.zeros((B, C, H, W), np.float32)
    for core in range(8):
        b, qc = divmod(core, QCH)
        out[b, :, ROWS * qc:ROWS * (qc + 1), :] = \
            res.results[core]["out"].reshape(C, ROWS, W)
    return out
